# revision 3
# baseline (speedup 1.0000x reference)
"""BilinearRelationNet Trainium2 kernel (8 NeuronCores, data-parallel over batch).

v2 layout strategy (feature-on-partitions, batch-on-free-dim):
  phase 1 (per tower t, per 512-row chunk): SWDGE casting DMA streams x fp32 ->
    fp16 with partition p holding 4 CONTIGUOUS rows (8KB contiguous reads,
    near-line-rate HBM); TensorE transposes 128x128 blocks (fp16 PSUM); DVE
    evacuates; TensorE mm1 (x@W1) accumulates fp32 PSUM; ACT evacuates h_pre
    to fp16 SBUF; DVE bn_stats on the fp16 copy.
  Per-tower stats AllGather: tower-0's collective and its BN+W2 pass overlap
    tower-1's phase 1; only tower-1's collective sits on the critical path.
  phase B (per tower): BN affine+relu (ACT/DVE) -> mm2 -> relu+b2 via
    activation bias on evacuation (no bias matmuls anywhere).
  phase C: elementwise combine (ACT/DVE/GPS) -> mm3 against W3 extended with
    ones columns (dot/n1/n2 row-sums ride along) -> relu+b3 -> mm4 -> stage.
  phase D: gather per-row scalars into [128, R/128] tiles, cosine+sigmoid
    finalization, DMA out.

The chunk DMA permutes rows (free position nb*128+p holds row 4p+nb); the
host applies the inverse permutation after gathering. b1 is dropped: BN
subtracts the batch mean of (x@W1 + b1), so b1 cancels exactly.
"""

import sys

sys.path.insert(0, "/opt/trn_rl_repo")

import numpy as np
import concourse.bass as bass
import concourse.bacc as bacc
import concourse.tile as tile
import concourse.mybir as mybir
from concourse import bass_utils

F32 = mybir.dt.float32
F16 = mybir.dt.float16
AF = mybir.ActivationFunctionType
ALU = mybir.AluOpType

N_CORES = 8
D = 512
H = 256
BN_EPS = 1e-5


def build_nc(n_chunks: int):
    """One SPMD program; each core handles R = n_chunks*512 rows of both x1/x2."""
    R = n_chunks * 512
    nc = bacc.Bacc("TRN2", target_bir_lowering=False, debug=False, num_devices=N_CORES)

    x_dram = [
        nc.dram_tensor("x1", [R, D], F32, kind="ExternalInput"),
        nc.dram_tensor("x2", [R, D], F32, kind="ExternalInput"),
    ]
    w1_d = nc.dram_tensor("w1p", [4, 128, H], F16, kind="ExternalInput")
    w2_d = nc.dram_tensor("w2p", [2, 128, 128], F16, kind="ExternalInput")
    w3_d = nc.dram_tensor("w3e", [5, 128, 67], F16, kind="ExternalInput")
    w4_d = nc.dram_tensor("w4p", [64, 1], F16, kind="ExternalInput")
    b2_d = nc.dram_tensor("b2c", [128, 1], F32, kind="ExternalInput")
    b3_d = nc.dram_tensor("b3c", [64, 1], F32, kind="ExternalInput")
    b4_d = nc.dram_tensor("b4c", [128, 1], F32, kind="ExternalInput")
    gamma_d = nc.dram_tensor("gamma2", [128, 2], F32, kind="ExternalInput")
    betabn_d = nc.dram_tensor("betabn2", [128, 2], F32, kind="ExternalInput")
    alpha_d = nc.dram_tensor("alphab", [128, 1], F32, kind="ExternalInput")
    beta_d = nc.dram_tensor("betab", [128, 1], F32, kind="ExternalInput")
    iden_d = nc.dram_tensor("iden", [128, 128], F16, kind="ExternalInput")
    out_d = nc.dram_tensor("out", [R], F32, kind="ExternalOutput")

    with tile.TileContext(nc) as tc:
        with (
            tc.tile_pool(name="const", bufs=1) as cpool,
            tc.tile_pool(name="persist", bufs=1) as hpool,
            tc.tile_pool(name="dram", bufs=1, space="DRAM") as dpool,
        ):
            # ---- constants to SBUF (identity first: transposes need it) ----
            idens = cpool.tile([128, 128], F16, tag="idens")
            nc.sync.dma_start(idens[:], iden_d[:])
            w1s = cpool.tile([128, 4 * H], F16, tag="w1s")
            for dc in range(4):
                nc.sync.dma_start(w1s[:, dc * H : (dc + 1) * H], w1_d[dc])
            w2s = cpool.tile([128, 2 * 128], F16, tag="w2s")
            for k in range(2):
                nc.sync.dma_start(w2s[:, k * 128 : (k + 1) * 128], w2_d[k])
            w3s = cpool.tile([128, 5 * 67], F16, tag="w3s")
            for k in range(5):
                nc.sync.dma_start(w3s[:, k * 67 : (k + 1) * 67], w3_d[k])
            w4s = cpool.tile([64, 1], F16, tag="w4s")
            nc.sync.dma_start(w4s[:], w4_d[:])
            b2s = cpool.tile([128, 1], F32, tag="b2s")
            nc.sync.dma_start(b2s[:], b2_d[:])
            b3s = cpool.tile([64, 1], F32, tag="b3s")
            nc.sync.dma_start(b3s[:], b3_d[:])
            b4s = cpool.tile([128, 1], F32, tag="b4s")
            nc.sync.dma_start(b4s[:], b4_d[:])
            gammas = cpool.tile([128, 2], F32, tag="gammas")
            nc.sync.dma_start(gammas[:], gamma_d[:])
            betabns = cpool.tile([128, 2], F32, tag="betabns")
            nc.sync.dma_start(betabns[:], betabn_d[:])
            alphas = cpool.tile([128, 1], F32, tag="alphas")
            nc.sync.dma_start(alphas[:], alpha_d[:])
            betas = cpool.tile([128, 1], F32, tag="betas")
            nc.sync.dma_start(betas[:], beta_d[:])

            # ---- persistent buffers ----
            # h_pre fp16, per (tower, h-chunk): [128, R]
            hp = [
                [hpool.tile([128, R], F16, tag=f"hp{t}{m}", name=f"hp{t}{m}") for m in range(2)]
                for t in range(2)
            ]
            # post-mm2 activations per tower: [128, R] fp16
            hh = [hpool.tile([128, R], F16, tag=f"hh{t}", name=f"hh{t}") for t in range(2)]
            # bn_stats accumulators per stat-set (t,m): [128, 6*n_chunks]
            sbstats = [
                hpool.tile([128, 6 * n_chunks], F32, tag=f"bst{s}", name=f"bst{s}") for s in range(4)
            ]
            # per-row scalar gather targets: dot, n1, n2, slearn
            ncols = R // 128
            tq = [hpool.tile([128, ncols], F32, tag=f"tq{q}", name=f"tq{q}") for q in range(4)]
            # BN scale/shift per tower: [128, 4] (scale_m at 2m, shift_m at 2m+1)
            gsh = [hpool.tile([128, 4], F32, tag=f"gsh{t}", name=f"gsh{t}") for t in range(2)]
            arin = [hpool.tile([128, 4], F32, tag=f"arin{t}", name=f"arin{t}") for t in range(2)]
            agout = [hpool.tile([128, 32], F32, tag=f"agout{t}", name=f"agout{t}") for t in range(2)]
            stw = [hpool.tile([128, 16], F32, tag=f"stw{t}", name=f"stw{t}") for t in range(2)]

            # ================= phase 1 + per-tower stats =================
            with (
                tc.tile_pool(name="p1sb", bufs=3) as p1,
                tc.tile_pool(name="p2sb", bufs=3) as p2,
                tc.tile_pool(name="p1ps", bufs=2, space="PSUM") as pp1,
                tc.tile_pool(name="p2ps", bufs=2, space="PSUM") as pp2,
            ):
                for t in range(2):
                    for c in range(n_chunks):
                        rows = slice(c * 512, (c + 1) * 512)
                        # partition p <- rows 4p..4p+3 (8KB contiguous per
                        # partition); free position nb*128+p <-> row 4p+nb
                        xfs = p1.tile([128, 4 * 512], F16, tag="xfs")
                        nc.gpsimd.dma_start(
                            xfs.rearrange("p (nb d) -> p nb d", nb=4),
                            x_dram[t][rows, :].rearrange("(p nb) d -> p nb d", nb=4),
                        )
                        xT = p1.tile([128, 4 * 512], F16, tag="xT")
                        for half in range(2):
                            ptr = pp1.tile([128, 1024], F16, tag="ptr")
                            for dci in range(2):
                                dc = half * 2 + dci
                                for nb in range(4):
                                    nc.tensor.transpose(
                                        ptr[:, dci * 512 + nb * 128 : dci * 512 + (nb + 1) * 128],
                                        xfs[:, nb * 512 + dc * 128 : nb * 512 + (dc + 1) * 128],
                                        idens[:],
                                    )
                            nc.vector.tensor_copy(
                                xT[:, half * 1024 : (half + 1) * 1024], ptr[:]
                            )
                        acc = pp1.tile([128, 1024], F32, tag="acc")
                        for m in range(2):
                            for dc in range(4):
                                nc.tensor.matmul(
                                    acc[:, m * 512 : (m + 1) * 512],
                                    w1s[:, dc * H + m * 128 : dc * H + (m + 1) * 128],
                                    xT[:, dc * 512 : (dc + 1) * 512],
                                    start=(dc == 0),
                                    stop=(dc == 3),
                                )
                        for m in range(2):
                            dst = hp[t][m][:, c * 512 : (c + 1) * 512]
                            nc.scalar.activation(
                                dst, acc[:, m * 512 : (m + 1) * 512], AF.Copy
                            )
                            nc.vector.bn_stats(
                                sbstats[t * 2 + m][:, c * 6 : (c + 1) * 6], dst
                            )

                    # ---- tower-t stats + AllGather ----
                    for m in range(2):
                        s = t * 2 + m
                        aggr = stw[t][:, m * 2 : m * 2 + 2]
                        nc.vector.bn_aggr(aggr, sbstats[s][:])
                        # arin[2m] = local mean ; arin[2m+1] = E[h^2]
                        nc.vector.tensor_copy(
                            arin[t][:, 2 * m : 2 * m + 1], aggr[:, 0:1]
                        )
                        msq = stw[t][:, 8 + m : 9 + m]
                        nc.vector.tensor_tensor(msq, aggr[:, 0:1], aggr[:, 0:1], ALU.mult)
                        nc.vector.tensor_tensor(
                            arin[t][:, 2 * m + 1 : 2 * m + 2], aggr[:, 1:2], msq, ALU.add
                        )
                    bnc_in = dpool.tile([128, 4], F32, name=f"bnc_in{t}")
                    bnc_out = dpool.tile(
                        [128 * N_CORES, 4], F32, addr_space="Shared", name=f"bnc_out{t}"
                    )
                    nc.sync.dma_start(bnc_in[:], arin[t][:])
                    nc.gpsimd.collective_compute(
                        "AllGather",
                        ALU.bypass,
                        ins=[bnc_in.opt()],
                        outs=[bnc_out.opt()],
                        replica_groups=[list(range(N_CORES))],
                    )
                    nc.sync.dma_start(
                        agout[t].rearrange("p (r j) -> p r j", r=N_CORES),
                        bnc_out.rearrange("(r p) j -> p r j", p=128),
                    )
                    # tree-sum 8 rank blocks of 4 cols -> sums [128, 4]
                    w = stw[t]
                    nc.vector.tensor_tensor(
                        w[:, 0:16], agout[t][:, 0:16], agout[t][:, 16:32], ALU.add
                    )
                    nc.vector.tensor_tensor(w[:, 0:8], w[:, 0:8], w[:, 8:16], ALU.add)
                    sums = w[:, 8:12]
                    nc.vector.tensor_tensor(sums, w[:, 0:4], w[:, 4:8], ALU.add)
                    # global mean/E[h^2] -> scale/shift  (vectorized over m)
                    gm = w[:, 0:2]
                    nc.vector.tensor_scalar(gm, sums[:, 0:4:2], 1.0 / N_CORES, None, ALU.mult)
                    gE = w[:, 2:4]
                    nc.vector.tensor_scalar(gE, sums[:, 1:4:2], 1.0 / N_CORES, None, ALU.mult)
                    gmsq = w[:, 4:6]
                    nc.vector.tensor_tensor(gmsq, gm, gm, ALU.mult)
                    var2 = w[:, 6:8]
                    nc.vector.tensor_tensor(var2, gE, gmsq, ALU.subtract)
                    vare = w[:, 12:14]
                    nc.vector.tensor_scalar(vare, var2, float(BN_EPS), None, ALU.add)
                    std2 = w[:, 14:16]
                    nc.scalar.activation(std2, vare, AF.Sqrt)
                    istd = w[:, 4:6]
                    nc.vector.reciprocal(istd, std2)
                    nc.vector.tensor_tensor(gsh[t][:, 0:4:2], istd, gammas[:], ALU.mult)
                    gmg = w[:, 6:8]
                    nc.vector.tensor_tensor(gmg, gm, gsh[t][:, 0:4:2], ALU.mult)
                    nc.vector.tensor_tensor(
                        gsh[t][:, 1:4:2], betabns[:], gmg, ALU.subtract
                    )

                # ================= phase B (per tower): BN+relu -> mm2 -> relu ====
                # Emitted after phase 1 so tower-0's pass fills engine gaps
                # during tower-1's phase 1 / AllGather latency.
                CW = 1024
                for t in range(2):
                    for c in range(R // CW):
                        cols = slice(c * CW, (c + 1) * CW)
                        hn0 = p2.tile([128, CW], F16, tag=f"hn0_{t}")
                        nc.scalar.activation(
                            hn0, hp[t][0][:, cols], AF.Relu,
                            scale=gsh[t][:, 0:1], bias=gsh[t][:, 1:2],
                        )
                        tmp = p2.tile([128, CW], F16, tag=f"hn1t_{t}")
                        nc.vector.tensor_scalar(
                            tmp, hp[t][1][:, cols],
                            gsh[t][:, 2:3], gsh[t][:, 3:4], ALU.mult, ALU.add,
                        )
                        hn1 = p2.tile([128, CW], F16, tag=f"hn1_{t}")
                        nc.vector.tensor_scalar(hn1, tmp, 0.0, None, ALU.max)
                        for hf in range(2):
                            hs = slice(hf * 512, (hf + 1) * 512)
                            pw = pp2.tile([128, 512], F32, tag="pw")
                            nc.tensor.matmul(
                                pw[:], w2s[:, 0:128], hn0[:, hs], start=True, stop=False
                            )
                            nc.tensor.matmul(
                                pw[:], w2s[:, 128:256], hn1[:, hs], start=False, stop=True
                            )
                            dst = hh[t][:, c * CW + hf * 512 : c * CW + (hf + 1) * 512]
                            if hf == 0:
                                nc.scalar.activation(dst, pw[:], AF.Relu, bias=b2s[:, 0:1])
                            else:
                                nc.vector.tensor_scalar(
                                    dst, pw[:], b2s[:, 0:1], 0.0, ALU.add, ALU.max
                                )

            # ================= phase C: combine -> mm3 -> mm4 =================
            with (
                tc.tile_pool(name="p3sb", bufs=3) as p3,
                tc.tile_pool(name="p3ps", bufs=2, space="PSUM") as pp3,
            ):
                CW = 1024
                for c in range(R // CW):
                    cols = slice(c * CW, (c + 1) * CW)
                    h0 = hh[0][:, cols]
                    h1 = hh[1][:, cols]
                    p_t = p3.tile([128, CW], F16, tag="p_t")
                    nc.vector.tensor_tensor(p_t[:], h0, h1, ALU.mult)
                    dd = p3.tile([128, CW], F16, tag="dd")
                    nc.vector.tensor_tensor(dd[:], h0, h1, ALU.subtract)
                    q_t = p3.tile([128, CW], F16, tag="q_t")
                    nc.scalar.activation(q_t[:], dd[:], AF.Abs)
                    r_t = p3.tile([128, CW], F16, tag="r_t")
                    nc.vector.tensor_tensor(r_t[:], h0, h1, ALU.add)
                    s1_t = p3.tile([128, CW], F16, tag="s1_t")
                    nc.scalar.activation(s1_t[:], h0, AF.Square)
                    s2_t = p3.tile([128, CW], F16, tag="s2_t")
                    nc.gpsimd.tensor_tensor(s2_t[:], h1, h1, ALU.mult)

                    rhs5 = [p_t, q_t, r_t, s1_t, s2_t]
                    stage = p3.tile([33, CW], F32, tag="stage")
                    r64 = p3.tile([64, CW], F16, tag="r64")
                    for hf in range(2):
                        hs = slice(hf * 512, (hf + 1) * 512)
                        pw3 = pp3.tile([128, 512], F32, tag="pw3")
                        for k in range(5):
                            nc.tensor.matmul(
                                pw3[0:67, :],
                                w3s[:, k * 67 : (k + 1) * 67],
                                rhs5[k][:, hs],
                                start=(k == 0),
                                stop=(k == 4),
                            )
                        nc.scalar.activation(
                            r64[:, hs], pw3[0:64, :], AF.Relu, bias=b3s[:, 0:1]
                        )
                        nc.vector.tensor_copy(stage[0:3, hs], pw3[64:67, :])
                        pw4 = pp3.tile([1, 512], F32, tag="pw4")
                        nc.tensor.matmul(
                            pw4[:], w4s[:], r64[:, hs], start=True, stop=True
                        )
                        nc.vector.tensor_copy(stage[32:33, hs], pw4[:])
                    ppc = CW // ncols  # partition rows of tq covered per block
                    for q in range(4):
                        sp = q if q < 3 else 32
                        nc.sync.dma_start(
                            tq[q][c * ppc : (c + 1) * ppc, :], stage[sp : sp + 1, :]
                        )

            # ================= phase D: finalize =================
            fin = hpool.tile([128, 6 * ncols], F32, tag="fin")

            def fcol(i):
                return fin[:, i * ncols : (i + 1) * ncols]

            nc.vector.tensor_tensor(fcol(0), tq[1][:], tq[2][:], ALU.mult)  # n1*n2
            nc.vector.tensor_scalar(fcol(2), fcol(0), 1e-30, None, ALU.add)
            nc.scalar.activation(fcol(1), fcol(2), AF.Sqrt)
            nc.vector.reciprocal(fcol(2), fcol(1))
            nc.vector.tensor_tensor(fcol(0), tq[0][:], fcol(2), ALU.mult)  # s_math
            nc.vector.tensor_scalar(fcol(1), fcol(0), 0.0, 1.0, ALU.max, ALU.min)
            nc.scalar.activation(fcol(3), tq[3][:], AF.Sigmoid, bias=b4s[:, 0:1])
            nc.vector.tensor_scalar(fcol(4), fcol(1), alphas[:, 0:1], None, ALU.mult)
            nc.vector.tensor_scalar(fcol(5), fcol(3), betas[:, 0:1], None, ALU.mult)
            nc.vector.tensor_tensor(fcol(0), fcol(4), fcol(5), ALU.add)
            nc.vector.tensor_scalar(fcol(1), fcol(0), 0.0, 1.0, ALU.max, ALU.min)
            nc.sync.dma_start(
                out_d.ap().rearrange("(p k) -> p k", p=128), fcol(1)
            )

    nc.compile()
    return nc


_NC_CACHE: dict = {}


def _get_nc(n_chunks):
    if n_chunks not in _NC_CACHE:
        _NC_CACHE[n_chunks] = build_nc(n_chunks)
    return _NC_CACHE[n_chunks]


def _prep_weights(W1, gamma, beta_bn, W2, b2, W3, b3, W4, b4, alpha, beta):
    f16 = np.float16
    f32 = np.float32
    W1 = np.asarray(W1, f32)
    W3 = np.asarray(W3, f32)
    w3e = np.zeros((5, 128, 67), f32)
    w3e[0, :, 0:64] = W3[0:128]
    w3e[1, :, 0:64] = W3[128:256]
    w3e[2, :, 0:64] = W3[256:384]
    w3e[0, :, 64] = 1.0  # dot = ones . (h1*h2)
    w3e[3, :, 65] = 1.0  # n1  = ones . h1^2
    w3e[4, :, 66] = 1.0  # n2  = ones . h2^2
    return {
        "w1p": np.ascontiguousarray(W1.reshape(4, 128, H).astype(f16)),
        "w2p": np.ascontiguousarray(np.asarray(W2, f32).reshape(2, 128, 128).astype(f16)),
        "w3e": w3e.astype(f16),
        "w4p": np.asarray(W4, f32).reshape(64, 1).astype(f16),
        "b2c": np.asarray(b2, f32).reshape(128, 1),
        "b3c": np.asarray(b3, f32).reshape(64, 1),
        "b4c": np.full((128, 1), np.asarray(b4, f32).reshape(-1)[0], f32),
        "gamma2": np.ascontiguousarray(np.asarray(gamma, f32).reshape(2, 128).T),
        "betabn2": np.ascontiguousarray(np.asarray(beta_bn, f32).reshape(2, 128).T),
        "alphab": np.full((128, 1), np.asarray(alpha, f32).reshape(-1)[0], f32),
        "betab": np.full((128, 1), np.asarray(beta, f32).reshape(-1)[0], f32),
        "iden": np.eye(128, dtype=f16),
    }


def _row_perm(R):
    """Device free-position -> source row, per 512-row chunk (see chunk DMA)."""
    j = np.arange(512)
    pos_of_row = (j % 4) * 128 + j // 4  # row -> device position
    return pos_of_row


def run_on_hw(x1, x2, weights, n_chunks, trace=False):
    R = n_chunks * 512
    nc = _get_nc(n_chunks)
    in_maps = []
    for c in range(N_CORES):
        m = {"x1": np.ascontiguousarray(x1[c * R : (c + 1) * R]),
             "x2": np.ascontiguousarray(x2[c * R : (c + 1) * R])}
        m.update(weights)
        in_maps.append(m)
    r = bass_utils.run_bass_kernel_spmd(
        nc, in_maps, core_ids=list(range(N_CORES)), trace=trace
    )
    pos = _row_perm(R)
    outs = []
    for c in range(N_CORES):
        dev = r.results[c]["out"].reshape(n_chunks, 512)
        outs.append(dev[:, pos].reshape(-1))  # out[row] = dev[pos_of_row]
    return np.concatenate(outs), r


def kernel(x1, x2, W1, b1, gamma, beta_bn, W2, b2, W3, b3, W4, b4, alpha, beta):
    x1 = np.asarray(x1, np.float32)
    x2 = np.asarray(x2, np.float32)
    n_chunks = x1.shape[0] // (N_CORES * 512)
    weights = _prep_weights(W1, gamma, beta_bn, W2, b2, W3, b3, W4, b4, alpha, beta)
    out, _ = run_on_hw(x1, x2, weights, n_chunks)
    return out.astype(np.float32)
